# revision 10
# baseline (speedup 1.0000x reference)
"""Multi-head causal attention (B=2, S=2048, E=1024, H=16, D=64) on 8 trn2 cores.

Sharding (Megatron-style): data-parallel over batch (2) x tensor-parallel over
heads (4 groups of 4 heads / 256 features). Core c: batch c//4, group c%4.

Host pre-converts x and weights to bf16. Per-core device program, pipelined
over 4 query chunks of 512:
  A(ic). xT for the chunk loaded straight from DRAM via DMA-transpose (XBAR)
  B(ic). qT/kT projections in [n, s] layout; v in natural [s, n] layout.
       PSUM evictions (+bias) on the Pool engine.
  C(ic). causal attention in transposed-score layout. Non-diagonal j-tiles are
       processed in PAIRS sharing one 2-bank PSUM tile and ONE exp activation
       (halves ACT instruction count so exp never paces the loop); diagonal
       tiles stay single with multiplicative 0/1 masking. Depth-2 software
       pipeline: ctx matmuls trail score matmuls by two units. Row 64 of the
       ctx psum = softmax denominator (v augmented with ones); normalization
       uses reciprocal_approx_fast + partition_broadcast + multiply.
  D(ic). AllGather the normalized ctxT chunk across the 4-core batch group
       (chunk 3 is gathered in two head-pair halves so the tail overlaps)
  E(ic). out[:, g*256:(g+1)*256] = ctxT_full.T @ Wo[:, slice] + bo[slice],
       emitted 1-2 chunks behind so collectives hide behind compute; chunk 3
       accumulates even ct-blocks (first AG half) then odd (second half).
Host only slices/converts inputs and concatenates the 8 disjoint out slices.
"""

import contextlib

import ml_dtypes
import numpy as np

import concourse.mybir as mybir
import concourse.tile as tile
from concourse import bacc
from concourse.bass_utils import run_bass_kernel_spmd

F32 = mybir.dt.float32
BF16 = mybir.dt.bfloat16

B, S, E, H, D = 2, 2048, 1024, 16, 64
N_CORES = 8
TP = 4                 # tensor-parallel degree (head groups per batch)
NSL = E // TP          # 256 features per core
HLOC = H // TP         # 4 heads per core
KT = E // 128          # 8 contraction tiles
IT = S // 128          # 16 sequence tiles
ICH = S // 512         # 4 sequence chunks of 512
SCALE = 1.0 / np.sqrt(D)

REPLICA_GROUPS = [[0, 1, 2, 3], [4, 5, 6, 7]]

_cache: dict = {}


def _emit(nc, tc, prm):
    x, wq, bq, wk, bk, wv, bv, wo, bo, mask, out = prm

    with contextlib.ExitStack() as stack:
        ent = stack.enter_context
        const = ent(tc.tile_pool(name="const", bufs=1))
        wpool = ent(tc.tile_pool(name="wpool", bufs=1))
        xt_p = ent(tc.tile_pool(name="xt", bufs=2))
        qkv_p = ent(tc.tile_pool(name="qkv", bufs=1))
        psum_s = ent(tc.tile_pool(name="psum_s", bufs=2, space="PSUM"))
        psum_mm = ent(tc.tile_pool(name="psum_mm", bufs=2, space="PSUM"))
        psum_c = ent(tc.tile_pool(name="psum_c", bufs=2, space="PSUM"))
        pwork = ent(tc.tile_pool(name="pwork", bufs=3))
        norm_p = ent(tc.tile_pool(name="norm", bufs=2))
        ctxt_p = ent(tc.tile_pool(name="ctxt", bufs=1))
        ctxf_p = ent(tc.tile_pool(name="ctxf", bufs=2))
        osb_p = ent(tc.tile_pool(name="osb", bufs=2))
        dram = ent(tc.tile_pool(name="dram", bufs=1, space="DRAM"))

        # ---- bias rows first on the SP queue (feed the Pool broadcasts,
        # which must precede the warmup collective) ----
        bq_sb = wpool.tile([128, 2], F32)
        bk_sb = wpool.tile([128, 2], F32)
        for b_sb, b_dr in ((bq_sb, bq), (bk_sb, bk)):
            nc.sync.dma_start(out=b_sb[:], in_=b_dr.rearrange("(t p) -> p t", p=128))
        bv_row = wpool.tile([1, NSL], F32)
        nc.sync.dma_start(out=bv_row[:], in_=bv[None, :])
        bo_row = wpool.tile([1, NSL], F32)
        nc.sync.dma_start(out=bo_row[:], in_=bo[None, :])
        bvb = wpool.tile([128, NSL], F32)
        nc.gpsimd.partition_broadcast(out_ap=bvb[:], in_ap=bv_row[:])
        bob = wpool.tile([128, NSL], F32)
        nc.gpsimd.partition_broadcast(out_ap=bob[:], in_ap=bo_row[:])

        # warmup collective early on the Pool queue: it blocks the issuing
        # queue until the cross-core rendezvous completes, so nothing that
        # feeds B(0)/C(0) may sit behind it
        warm_in = dram.tile([1, 128], F32)
        warm_out = dram.tile([TP, 1, 128], F32)
        nc.gpsimd.collective_compute(
            "AllGather", mybir.AluOpType.bypass,
            replica_groups=REPLICA_GROUPS,
            ins=[warm_in.opt()], outs=[warm_out.opt()],
        )

        # ---- xT chunk loads: DMA-transpose straight from DRAM (bf16) ----
        xt_tiles = {}

        def stage_a(ic):
            xt_sb = xt_p.tile([128, KT, 512], BF16, tag="xt")
            for kt in range(KT):
                nc.sync.dma_start_transpose(
                    out=xt_sb[:, kt, :],
                    in_=x[ic * 512:(ic + 1) * 512, kt * 128:(kt + 1) * 128])
            xt_tiles[ic] = xt_sb

        stage_a(0)

        # ---- weights (bf16 in DRAM): wq/wk/wv + mask issue from the ACT
        # queue (idle until the first eviction), wo from SP (needed late) ----
        wq_sb = wpool.tile([128, KT, NSL], BF16)
        wk_sb = wpool.tile([128, KT, NSL], BF16)
        wv_sb = wpool.tile([128, KT, NSL], BF16)
        wo_sb = wpool.tile([128, KT, NSL], BF16)
        for w_sb, w_dr in ((wq_sb, wq), (wk_sb, wk), (wv_sb, wv)):
            w_r = w_dr.rearrange("(t p) n -> p t n", p=128)
            for kt in range(KT):
                nc.scalar.dma_start(out=w_sb[:, kt, :], in_=w_r[:, kt, :])
        mask_sb = const.tile([128, 128], BF16)
        nc.scalar.dma_start(out=mask_sb[:], in_=mask[:])
        stage_a(1)
        wo_r = wo.rearrange("(t p) n -> p t n", p=128)
        for kt in range(KT):
            nc.sync.dma_start(out=wo_sb[:, kt, :], in_=wo_r[:, kt, :])

        # ---- persistent activations ----
        qt_sb = qkv_p.tile([128, 2, S], BF16)
        kt_sb = qkv_p.tile([128, 2, S], BF16)
        v_sb = qkv_p.tile([128, IT, HLOC, D + 1], BF16)
        ones_col = qkv_p.tile([128, IT, HLOC, 1], F32)
        nc.vector.memset(ones_col[:], 1.0)
        nc.vector.tensor_copy(v_sb[:, :, :, D:D + 1], ones_col[:])
        ctxt_sb = ctxt_p.tile([128, 2, S], BF16)

        # DRAM bounce buffers: whole-chunk allgathers for ic<3, two head-pair
        # halves for ic=3 so the tail collective overlaps stage E
        cc_in = [dram.tile([2 * 128, 512], BF16, name=f"cc_in{ic}")
                 for ic in range(ICH - 1)]
        cc_out = [dram.tile([TP, 2 * 128, 512], BF16, name=f"cc_out{ic}")
                  for ic in range(ICH - 1)]
        cc_in_h = [dram.tile([128, 512], BF16, name=f"cc_in_h{k}")
                   for k in range(2)]
        cc_out_h = [dram.tile([TP, 128, 512], BF16, name=f"cc_out_h{k}")
                    for k in range(2)]

        def stage_b(ic):
            xt_sb = xt_tiles[ic]
            osl = slice(ic * 512, (ic + 1) * 512)
            for (w_sb, b_sb, o_sb) in ((wq_sb, bq_sb, qt_sb),
                                       (wk_sb, bk_sb, kt_sb)):
                for nt in range(2):
                    pm = psum_mm.tile([128, 512], F32, tag="pqk")
                    for kt in range(KT):
                        nc.tensor.matmul(
                            pm[:],
                            w_sb[:, kt, nt * 128:(nt + 1) * 128],
                            xt_sb[:, kt, :],
                            start=(kt == 0), stop=(kt == KT - 1),
                        )
                    nc.scalar.activation(
                        out=o_sb[:, nt, osl], in_=pm[:],
                        func=mybir.ActivationFunctionType.Identity,
                        bias=b_sb[:, nt:nt + 1])
            for k4, it in enumerate(range(4 * ic, 4 * ic + 4)):
                pv = psum_mm.tile([128, NSL], F32, tag="pqk")
                for kt in range(KT):
                    nc.tensor.matmul(
                        pv[:],
                        xt_sb[:, kt, k4 * 128:(k4 + 1) * 128],
                        wv_sb[:, kt, :],
                        start=(kt == 0), stop=(kt == KT - 1),
                    )
                nc.vector.tensor_add(
                    out=v_sb[:, it, :, 0:D],
                    in0=pv[:].rearrange("p (h d) -> p h d", d=D),
                    in1=bvb[:].rearrange("p (h d) -> p h d", d=D))

        def stage_c(ic, fillers=None):
            i0 = ic * 512
            njt = 4 * (ic + 1)

            for h in range(HLOC):
                nt, base = divmod(h, 2)
                base *= D
                pc = psum_c.tile([D + 1, 512], F32, tag="pc")

                # units: non-diagonal jt pairs, then the 4 diagonal singles
                units = [("pair", 2 * p, 2 * p + 1) for p in range(2 * ic)]
                units += [("diag", jt, None) for jt in range(4 * ic, njt)]

                def emit_scores(u):
                    kind, jta, jtb = u
                    if kind == "pair":
                        ps = psum_s.tile([128, 1024], F32, tag="ps")
                        for half, jt in ((0, jta), (1, jtb)):
                            nc.tensor.matmul(
                                ps[:, half * 512:(half + 1) * 512],
                                kt_sb[base:base + D, nt,
                                      jt * 128:(jt + 1) * 128],
                                qt_sb[base:base + D, nt, i0:i0 + 512],
                                start=True, stop=True,
                            )
                        pw = pwork.tile([128, 1024], BF16, tag="pwp")
                        nc.scalar.activation(
                            out=pw[:], in_=ps[:],
                            func=mybir.ActivationFunctionType.Exp,
                            scale=float(SCALE))
                        return pw, 0
                    jt = jta
                    c0 = (jt - 4 * ic) * 128
                    ps = psum_s.tile([128, 1024], F32, tag="ps")
                    nc.tensor.matmul(
                        ps[:, c0:512],
                        kt_sb[base:base + D, nt, jt * 128:(jt + 1) * 128],
                        qt_sb[base:base + D, nt, i0 + c0:i0 + 512],
                        start=True, stop=True,
                    )
                    pw = pwork.tile([128, 512], BF16, tag="pws")
                    nc.scalar.activation(
                        out=pw[:, c0:], in_=ps[:, c0:512],
                        func=mybir.ActivationFunctionType.Exp,
                        scale=float(SCALE))
                    nc.vector.tensor_mul(
                        pw[:, c0:c0 + 128], pw[:, c0:c0 + 128], mask_sb[:])
                    return pw, c0

                def emit_ctx(u, pw_c0):
                    kind, jta, jtb = u
                    pw, c0 = pw_c0
                    if kind == "pair":
                        for half, jt in ((0, jta), (1, jtb)):
                            nc.tensor.matmul(
                                pc[:],
                                v_sb[:, jt, h, :],
                                pw[:, half * 512:(half + 1) * 512],
                                start=(jt == 0), stop=(jt == njt - 1),
                            )
                    else:
                        jt = jta
                        nc.tensor.matmul(
                            pc[:, c0:],
                            v_sb[:, jt, h, :],
                            pw[:, c0:],
                            start=(jt == 0), stop=(jt == njt - 1),
                        )

                # depth-2 software pipeline: ctx trails scores by two units
                pend = []
                for u in units:
                    pend.append((u, emit_scores(u)))
                    if len(pend) > 2:
                        uu, pp = pend.pop(0)
                        emit_ctx(uu, pp)
                for uu, pp in pend:
                    emit_ctx(uu, pp)

                # normalize: 1/l (approx) broadcast down 64 partitions
                lsum = norm_p.tile([1, 512], F32, tag="lsum")
                nc.vector.tensor_copy(lsum[:], pc[D:D + 1, :])
                lrow = norm_p.tile([1, 512], F32, tag="lrow")
                nc.vector.reciprocal_approx_fast(out=lrow[:], in_=lsum[:])
                lb = norm_p.tile([D, 512], F32, tag="lb")
                nc.gpsimd.partition_broadcast(out_ap=lb[:], in_ap=lrow[:])
                nc.vector.tensor_mul(
                    ctxt_sb[base:base + D, nt, i0:i0 + 512],
                    pc[0:D, :], lb[:])

                # ship finished head-pair rows to the collective bounce buffer
                if h == 1 or h == 3:
                    half = h // 2
                    src = ctxt_sb[:, half, i0:i0 + 512]
                    if ic < ICH - 1:
                        nc.gpsimd.dma_start(
                            out=cc_in[ic][half * 128:(half + 1) * 128, :],
                            in_=src)
                        if h == 3:
                            nc.gpsimd.collective_compute(
                                "AllGather", mybir.AluOpType.bypass,
                                replica_groups=REPLICA_GROUPS,
                                ins=[cc_in[ic].opt()], outs=[cc_out[ic].opt()],
                            )
                    else:
                        nc.gpsimd.dma_start(out=cc_in_h[half][:], in_=src)
                        nc.gpsimd.collective_compute(
                            "AllGather", mybir.AluOpType.bypass,
                            replica_groups=REPLICA_GROUPS,
                            ins=[cc_in_h[half].opt()],
                            outs=[cc_out_h[half].opt()],
                        )

                for f in (fillers or {}).get(h, []):
                    f()

        def stage_e_load(ic):
            ctxf_sb = ctxf_p.tile([128, KT, 512], BF16, tag="ctxf")
            cc_r = cc_out[ic].rearrange("g (t p) i -> p (g t) i", p=128)
            for k in range(4):
                nc.sync.dma_start(
                    out=ctxf_sb[:, :, k * 128:(k + 1) * 128],
                    in_=cc_r[:, :, k * 128:(k + 1) * 128])
            return ctxf_sb

        def stage_e_part(ic, ctxf_sb, k):
            it = 4 * ic + k
            po = psum_mm.tile([128, NSL], F32, tag="pqk")
            for ct in range(KT):
                nc.tensor.matmul(
                    po[:],
                    ctxf_sb[:, ct, k * 128:(k + 1) * 128],
                    wo_sb[:, ct, :],
                    start=(ct == 0), stop=(ct == KT - 1),
                )
            ot = osb_p.tile([128, NSL], F32, tag="ot")
            nc.vector.tensor_add(out=ot[:], in0=po[:], in1=bob[:])
            nc.gpsimd.dma_start(
                out=out[it * 128:(it + 1) * 128, :], in_=ot[:])

        def tail_load(half):
            cf = ctxf_p.tile([128, TP, 512], BF16, tag=f"ctxf3{half}",
                             name=f"cf{half}")
            nc.sync.dma_start(
                out=cf[:], in_=cc_out_h[half].rearrange("g p i -> p g i"))
            return cf

        def stage_e_tail(ctxf_h):
            # chunk 3: even ct-blocks come from the first AG half, odd from
            # the second; accumulate evens while the second half transfers.
            # One psum BANK per k-block chain (start=True re-inits the bank).
            ic = ICH - 1
            pos = [psum_s.tile([128, 1024], F32, tag="ps", name=f"po{i}")
                   for i in range(2)]
            for k in range(4):
                po = pos[k // 2][:, (k % 2) * 512:(k % 2) * 512 + NSL]
                for s in range(TP):
                    nc.tensor.matmul(
                        po,
                        ctxf_h[0][:, s, k * 128:(k + 1) * 128],
                        wo_sb[:, 2 * s, :],
                        start=(s == 0), stop=False,
                    )
            for k in range(4):
                po = pos[k // 2][:, (k % 2) * 512:(k % 2) * 512 + NSL]
                for s in range(TP):
                    nc.tensor.matmul(
                        po,
                        ctxf_h[1][:, s, k * 128:(k + 1) * 128],
                        wo_sb[:, 2 * s + 1, :],
                        start=False, stop=(s == TP - 1),
                    )
                it = 4 * ic + k
                ot = osb_p.tile([128, NSL], F32, tag="ot")
                nc.vector.tensor_add(out=ot[:], in0=po, in1=bob[:])
                nc.gpsimd.dma_start(
                    out=out[it * 128:(it + 1) * 128, :], in_=ot[:])

        # ---- pipeline ----
        # E(0) runs inside C(2) (its allgather is delayed by the warmup
        # rendezvous); E(1)/E(2) split across C(3); E(3) in the tail.
        stage_b(0)
        stage_c(0)
        stage_a(2)
        stage_b(1)
        stage_c(1)
        stage_a(3)
        stage_b(2)
        ctxf0 = stage_e_load(0)
        fillers = {h: [lambda icc=0, cf=ctxf0, kk=h: stage_e_part(icc, cf, kk)]
                   for h in range(HLOC)}
        stage_c(2, fillers)
        stage_b(3)
        ctxf1 = stage_e_load(1)
        ctxf2 = stage_e_load(2)
        ctxf_h = []
        fillers = {}
        for h in range(HLOC):
            icc, cf = (1, ctxf1) if h < 2 else (2, ctxf2)
            ks = (2 * h % 4, 2 * h % 4 + 1)
            fillers[h] = [
                lambda icc=icc, cf=cf, kk=k: stage_e_part(icc, cf, kk)
                for k in ks]
        # ctxf loads for the tail right after each half's allgather trigger
        fillers[1].append(lambda: ctxf_h.append(tail_load(0)))
        fillers[3].append(lambda: ctxf_h.append(tail_load(1)))
        stage_c(3, fillers)
        stage_e_tail(ctxf_h)


def _build():
    nc = bacc.Bacc("TRN2", target_bir_lowering=False, debug=False,
                   num_devices=N_CORES)
    x = nc.declare_dram_parameter("x", [S, E], BF16, isOutput=False).ap()
    wq = nc.declare_dram_parameter("wq", [E, NSL], BF16, isOutput=False).ap()
    bq = nc.declare_dram_parameter("bq", [NSL], F32, isOutput=False).ap()
    wk = nc.declare_dram_parameter("wk", [E, NSL], BF16, isOutput=False).ap()
    bk = nc.declare_dram_parameter("bk", [NSL], F32, isOutput=False).ap()
    wv = nc.declare_dram_parameter("wv", [E, NSL], BF16, isOutput=False).ap()
    bv = nc.declare_dram_parameter("bv", [NSL], F32, isOutput=False).ap()
    wo = nc.declare_dram_parameter("wo", [E, NSL], BF16, isOutput=False).ap()
    bo = nc.declare_dram_parameter("bo", [NSL], F32, isOutput=False).ap()
    mask = nc.declare_dram_parameter("mask", [128, 128], BF16,
                                     isOutput=False).ap()
    out = nc.declare_dram_parameter("out", [S, NSL], F32, isOutput=True).ap()

    with tile.TileContext(nc) as tc:
        _emit(nc, tc, (x, wq, bq, wk, bk, wv, bv, wo, bo, mask, out))
    nc.compile()
    return nc


def _mask():
    jl = np.arange(128, dtype=np.int64)[:, None]
    il = np.arange(128, dtype=np.int64)[None, :]
    return np.ascontiguousarray((il >= jl).astype(ml_dtypes.bfloat16))


def kernel(x, Wq, bq, Wk, bk, Wv, bv, Wo, bo, _trace=False, _trace_cores=None):
    if "nc" not in _cache:
        _cache["nc"] = _build()
    nc = _cache["nc"]
    mask = _mask()
    x = np.asarray(x, dtype=np.float32)
    bf = ml_dtypes.bfloat16
    in_maps = []
    for c in range(N_CORES):
        bi, g = divmod(c, TP)
        sl = slice(g * NSL, (g + 1) * NSL)
        in_maps.append({
            "x": np.ascontiguousarray(x[bi].astype(bf)),
            "wq": np.ascontiguousarray(np.asarray(Wq, np.float32)[:, sl].astype(bf)),
            "bq": np.ascontiguousarray(np.asarray(bq, np.float32)[sl]),
            "wk": np.ascontiguousarray(np.asarray(Wk, np.float32)[:, sl].astype(bf)),
            "bk": np.ascontiguousarray(np.asarray(bk, np.float32)[sl]),
            "wv": np.ascontiguousarray(np.asarray(Wv, np.float32)[:, sl].astype(bf)),
            "bv": np.ascontiguousarray(np.asarray(bv, np.float32)[sl]),
            "wo": np.ascontiguousarray(np.asarray(Wo, np.float32)[:, sl].astype(bf)),
            "bo": np.ascontiguousarray(np.asarray(bo, np.float32)[sl]),
            "mask": mask,
        })
    res = run_bass_kernel_spmd(
        nc, in_maps, list(range(N_CORES)),
        trace=_trace, trace_cores=_trace_cores)
    out = np.empty((B, S, E), np.float32)
    for c in range(N_CORES):
        bi, g = divmod(c, TP)
        out[bi, :, g * NSL:(g + 1) * NSL] = res.results[c]["out"]
    if _trace:
        _cache["last_result"] = res
    return out
